# revision 22
# baseline (speedup 1.0000x reference)
"""Trainium2 kernel for nn_Linear_14912126452257 (scatter_memory).

Computes: new_weight = weight + scatter_add(shira_indices, shira_weight);
          out = x @ new_weight^T + bias

Sharding: column-parallel over out_features across 8 NeuronCores
(each core owns 512 of 4096 output features). x is replicated; the
sparse COO entries are partitioned by owning row-shard.

Per-core device algorithm:
  1. Scatter: COO entries, bucketed by (c//128, r_loc//128) and padded
     to 128-entry tiles, arrive as host-marshalled one-hot tile pairs in
     fp8-e3m4 (vcoh[j, c%128] = v*64, roh[j, r%128] = 1).  The PE
     accumulates delta^T[c, o] = vcoh^T @ roh into PSUM (duplicates add
     natively); DVE then fuses descale+add: W'^T = pd/64 + W^T, bf16.
  2. GEMM: out[m, o] = sum_ic xT[ic]^T @ W'^T[ic] in bf16 with fp32
     PSUM accumulation, + bias epilogue on DVE.  The first six GEMM
     m-tiles are interleaved into the scatter loop (accumulating chunk
     by chunk as W' chunks appear) so the PE stays busy while the
     one-hot tiles stream in from HBM.
Host only marshals data (transpose/cast/bucket/pad/one-hot expand) and
concatenates the per-core output shards.
"""

import sys

for _p in ("/opt/trn_rl_repo", "/root/.axon_site/_ro/trn_rl_repo"):
    if _p not in sys.path:
        sys.path.append(_p)

import numpy as np
import ml_dtypes

import concourse.bass as bass
import concourse.mybir as mybir
import concourse.tile as tile
from concourse.bass_utils import run_bass_kernel_spmd

P = 128
IN_F = 4096
OUT_F = 4096
N_CORES = 8
O_SHARD = OUT_F // N_CORES  # 512
NK = IN_F // P  # 32 contraction chunks
NOC = O_SHARD // P  # 4 output sub-chunks per core
M_TOT = 8192  # 4 * 2048 tokens
SUPER_M = 512  # tokens per x super-tile
NSUP = M_TOT // SUPER_M
MT_PER_SUP = SUPER_M // P
SCALING = 1.0
VSCALE = 64.0  # fp8-e3m4 value pre-scale (min normal 2^-2; v ~ 0.02)
N_EARLY = 6  # GEMM m-tiles interleaved into the scatter loop


def _build_bass(bucket_tiles):
    """Build the SPMD Bass program. bucket_tiles[ic][oc] = number of
    128-entry one-hot tile pairs for bucket (ic, oc); same for every
    core (padded)."""
    ic_tiles = [int(sum(bucket_tiles[ic])) for ic in range(NK)]
    t_total = int(sum(ic_tiles))
    nc = bass.Bass("TRN2", target_bir_lowering=False, debug=False, num_devices=1)

    # x pre-packed per supertile so a supertile load is one contiguous
    # 32KB run per partition (128 DMA descriptors, not 4096)
    xt_d = nc.dram_tensor("xt", [P, NSUP, NK, SUPER_M], mybir.dt.bfloat16, kind="ExternalInput").ap()
    wt_d = nc.dram_tensor("wt", [P, NK, O_SHARD], mybir.dt.bfloat16, kind="ExternalInput").ap()
    bias_d = nc.dram_tensor("bias", [P, O_SHARD], mybir.dt.float32, kind="ExternalInput").ap()
    # one-hot tile pairs: [:, t, 0:128] = vcoh (values*64), [:, t, 128:256] = roh
    oh_d = nc.dram_tensor("oh", [P, t_total, 2 * P], mybir.dt.float8e3, kind="ExternalInput").ap()
    out_d = nc.dram_tensor("out", [M_TOT, O_SHARD], mybir.dt.float32, kind="ExternalOutput").ap()

    out_t = out_d.rearrange("(mt p) o -> mt p o", p=P)

    with tile.TileContext(nc) as tc:
        with (
            tc.tile_pool(name="persist", bufs=1) as persist,
            tc.tile_pool(name="work", bufs=3) as work,
            tc.tile_pool(name="ohpool", bufs=3) as ohpool,
            tc.tile_pool(name="xpool", bufs=3) as xpool,
        ):
            wt_bf = persist.tile([P, NK, O_SHARD], mybir.dt.bfloat16)
            wt_sb = persist.tile([P, NK, O_SHARD], mybir.dt.bfloat16)
            bias_sb = persist.tile([P, O_SHARD], mybir.dt.float32)
            zeros = persist.tile([P, O_SHARD], mybir.dt.bfloat16)
            nc.vector.memset(zeros[:], 0.0)
            nc.sync.dma_start(bias_sb[:], bias_d[:])
            nc.sync.dma_start(wt_sb[:], wt_d[:])

            def load_sup(sup, defer=False):
                xsb = xpool.tile([P, NK, SUPER_M], mybir.dt.bfloat16, tag="xsb")
                if not defer:
                    nc.sync.dma_start(xsb[:], xt_d[:, sup])
                return xsb

            def load_sup_chunk(xsb, sup, ic):
                nc.sync.dma_start(xsb[:, ic, :], xt_d[:, sup, ic, :])

            def epilogue(po, sup, mt):
                osb = work.tile([P, O_SHARD], mybir.dt.float32, tag="osb")
                nc.vector.tensor_tensor(
                    out=osb[:], in0=po[:], in1=bias_sb[:], op=mybir.AluOpType.add
                )
                nc.scalar.dma_start(out_t[sup * MT_PER_SUP + mt], osb[:])

            xsb0 = load_sup(0, defer=True)
            xsb1 = load_sup(1, defer=True)
            early_src = [
                (xsb0, 0, 0), (xsb0, 0, 1), (xsb0, 0, 2), (xsb0, 0, 3),
                (xsb1, 1, 0), (xsb1, 1, 1),
            ][:N_EARLY]

            # ---- opening: scatter W' chunks, with early GEMM interleaved ----
            # single PSUM pool for the whole kernel: tags pd(x2) + early0..5
            # cover all 8 banks; the main GEMM rotates over the same tags so
            # bank reuse carries exact deps (no pool-release barriers).
            with tc.tile_pool(name="psum", bufs=1, space="PSUM") as psum_pool:
                early = [
                    psum_pool.tile([P, O_SHARD], mybir.dt.float32, name=f"early{k}", tag=f"early{k}")
                    for k in range(N_EARLY)
                ]
                def early_mms(ic):
                    # early GEMM matmuls for chunk ic (lagged one chunk so
                    # the DVE chunk-assembly overlaps PE work)
                    for k, (xsb, _, mt) in enumerate(early_src):
                        nc.tensor.matmul(
                            out=early[k][:],
                            lhsT=xsb[:, ic, mt * P : (mt + 1) * P],
                            rhs=wt_bf[:, ic, :],
                            start=(ic == 0), stop=(ic == NK - 1),
                        )

                # PE warmup: dummy matmuls with no DMA deps keep the PE
                # busy through the DMA-queue startup window so the HAM
                # clock gate reaches 8/8 before real work arrives.
                wup = psum_pool.tile([P, O_SHARD], mybir.dt.float32, tag="pd", bufs=2)
                for _ in range(60):
                    nc.tensor.matmul(
                        out=wup[:], lhsT=zeros[:, 0:P], rhs=zeros[:],
                        start=True, stop=True,
                    )

                tbase = 0
                for ic2 in range(0, NK, 2):
                    # batched loads for a PAIR of chunks: bigger contiguous
                    # runs lift the opening DMA rate
                    ntp = ic_tiles[ic2] + ic_tiles[ic2 + 1]
                    ohc = ohpool.tile([P, ntp, 2 * P], mybir.dt.float8e3, tag="ohc")
                    nc.sync.dma_start(ohc[:], oh_d[:, tbase : tbase + ntp, :])
                    nc.sync.dma_start(
                        xsb0[:, ic2 : ic2 + 2, :], xt_d[:, 0, ic2 : ic2 + 2, :]
                    )
                    nc.sync.dma_start(
                        xsb1[:, ic2 : ic2 + 2, :], xt_d[:, 1, ic2 : ic2 + 2, :]
                    )
                    t = 0
                    for ic in (ic2, ic2 + 1):
                        pd = psum_pool.tile([P, O_SHARD], mybir.dt.float32, tag="pd", bufs=2)
                        for oc in range(NOC):
                            ntoc = int(bucket_tiles[ic][oc])
                            for j in range(ntoc):
                                nc.tensor.matmul(
                                    out=pd[:, oc * P : (oc + 1) * P],
                                    lhsT=ohc[:, t, 0:P],
                                    rhs=ohc[:, t, P : 2 * P],
                                    start=(j == 0), stop=(j == ntoc - 1),
                                )
                                t += 1
                        # W'^T chunk = pd/VSCALE + W^T chunk, cast bf16
                        nc.vector.scalar_tensor_tensor(
                            out=wt_bf[:, ic, :],
                            in0=pd[:],
                            scalar=1.0 / VSCALE,
                            in1=wt_sb[:, ic, :],
                            op0=mybir.AluOpType.mult,
                            op1=mybir.AluOpType.add,
                        )
                        if ic >= 2:
                            early_mms(ic - 2)
                    tbase += ntp
                # transition cover: run the first main tile's mms for chunks
                # 0..NK-3 between the tail early-mm groups so the PE is never
                # idle while stt(30)/stt(31) complete on DVE
                trans_po = psum_pool.tile(
                    [P, O_SHARD], mybir.dt.float32, name="trans_po", tag="pd", bufs=2
                )
                trans_src = (xsb1, 1, 2)
                for ic in range(NK - 2):
                    nc.tensor.matmul(
                        out=trans_po[:],
                        lhsT=trans_src[0][:, ic, trans_src[2] * P : (trans_src[2] + 1) * P],
                        rhs=wt_bf[:, ic, :],
                        start=(ic == 0), stop=False,
                    )
                early_mms(NK - 2)
                early_mms(NK - 1)
                for ic in (NK - 2, NK - 1):
                    nc.tensor.matmul(
                        out=trans_po[:],
                        lhsT=trans_src[0][:, ic, trans_src[2] * P : (trans_src[2] + 1) * P],
                        rhs=wt_bf[:, ic, :],
                        start=False, stop=(ic == NK - 1),
                    )
                epilogue(trans_po, trans_src[1], trans_src[2])

                # ---- main GEMM: remaining m-tiles ----
                tag_seq = ["pd", "pd"] + [f"early{k}" for k in range(N_EARLY)]
                tile_ctr = [0]

                def gemm_tile(xsb, sup, mt):
                    tag = tag_seq[tile_ctr[0] % len(tag_seq)]
                    tile_ctr[0] += 1
                    po = psum_pool.tile(
                        [P, O_SHARD], mybir.dt.float32, name="po", tag=tag,
                        bufs=2 if tag == "pd" else 1,
                    )
                    for ic in range(NK):
                        nc.tensor.matmul(
                            out=po[:],
                            lhsT=xsb[:, ic, mt * P : (mt + 1) * P],
                            rhs=wt_bf[:, ic, :],
                            start=(ic == 0), stop=(ic == NK - 1),
                        )
                    epilogue(po, sup, mt)

                # interleave the early-tile epilogues between the first main
                # tiles: main tiles grab the freed psum_d banks first, so the
                # PE never waits on the epilogue chain at the phase switch
                # 1:1 interleave of the next tiles with early epilogues so
                # each early bank is drained just before its tag is reused
                done = {(s2, m2) for (_, s2, m2) in early_src} | {(1, 2)}
                rest = [(xsb1, 1, mt) for mt in range(MT_PER_SUP) if (1, mt) not in done]
                xsb2 = load_sup(2)
                rest += [(xsb2, 2, mt) for mt in range(MT_PER_SUP)]
                epi_iter = iter(range(N_EARLY))
                for xsb, sup, mt in rest:
                    gemm_tile(xsb, sup, mt)
                    k = next(epi_iter, None)
                    if k is not None:
                        epilogue(early[k], early_src[k][1], early_src[k][2])
                for k in epi_iter:
                    epilogue(early[k], early_src[k][1], early_src[k][2])
                for sup in range(3, NSUP):
                    xsb = load_sup(sup)
                    for mt in range(MT_PER_SUP):
                        gemm_tile(xsb, sup, mt)
    return nc


def _split_multi_waits(nc):
    """Walrus in this container rejects compute-engine instructions carrying
    more than one sync wait (setupSyncWait: 'Too many sync wait commands').
    Hoist all-but-none of each such instruction's waits onto standalone
    EventSemaphore (pure wait) instructions inserted just before it in the
    same engine stream — semantically identical, per-engine order preserved."""
    import concourse.mybir as mybir

    n_split = 0
    for fn in nc.m.functions:
        for block in fn.blocks:
            new_instructions = []
            for inst in block.instructions:
                si = getattr(inst, "sync_info", None)
                waits = list(si.on_wait) if si is not None else []
                if len(waits) > 1:
                    for w in waits:
                        n_split += 1
                        new_instructions.append(
                            mybir.InstEventSemaphore(
                                name=f"{inst.name}-w{n_split}",
                                engine=inst.engine,
                                ins=[],
                                outs=[],
                                sync_info=mybir.SyncInfo(
                                    on_wait=[w], on_update=[]
                                ),
                            )
                        )
                    inst.sync_info = mybir.SyncInfo(
                        on_wait=[], on_update=list(si.on_update)
                    )
                new_instructions.append(inst)
            block.instructions = new_instructions
    return n_split


def _prep_inputs(x, weight, bias, shira_weight, shira_indices):
    """Host-side marshalling: transpose/cast x, shard+transpose W, bucket
    the COO entries by (core, c//128, r_loc//128), pad to 128-entry tiles
    and expand into fp8 one-hot tile pairs."""
    x2 = np.asarray(x, dtype=np.float32).reshape(M_TOT, IN_F)
    # [P, NSUP, NK, SUPER_M]: per-(partition, supertile) contiguous runs
    xt = np.ascontiguousarray(
        x2.T.reshape(NK, P, NSUP, SUPER_M).transpose(1, 2, 0, 3)
    ).astype(ml_dtypes.bfloat16)

    w = np.asarray(weight, dtype=np.float32)
    bias_np = np.asarray(bias, dtype=np.float32)
    rows = np.asarray(shira_indices[0]).astype(np.int64)
    cols = np.asarray(shira_indices[1]).astype(np.int64)
    vals = (np.asarray(shira_weight, dtype=np.float32) * SCALING * VSCALE).astype(
        ml_dtypes.float8_e3m4
    )

    core = rows // O_SHARD
    r_loc = rows % O_SHARD
    oc = r_loc // P
    r128 = r_loc % P
    ic = cols // P
    c128 = cols % P

    # bucket = (core, ic, oc); counts per bucket
    NB = NK * NOC
    bucket = ic * NOC + oc  # 0..127 within a core
    counts = np.zeros((N_CORES, NB), dtype=np.int64)
    np.add.at(counts, (core, bucket), 1)
    # padded tiles per bucket: max across cores, at least 1
    bt_flat = np.maximum(1, -(-counts.max(axis=0) // P))  # [NB]
    bucket_tiles = bt_flat.reshape(NK, NOC)
    t_total = int(bt_flat.sum())
    tile_base = np.concatenate([[0], np.cumsum(bt_flat)[:-1]])  # [NB]

    # sort entries by (core, bucket); rank within segment
    key = core * NB + bucket
    order = np.argsort(key, kind="stable")
    key_s = key[order]
    seg_starts = np.searchsorted(key_s, np.arange(N_CORES * NB))
    rank = np.arange(len(order)) - np.repeat(
        seg_starts, np.diff(np.concatenate([seg_starts, [len(order)]]))
    )
    core_s = core[order]
    b_s = bucket[order]
    # global entry slot: tile t = tile_base[b] + rank//P, row p = rank%P
    t_idx = tile_base[b_s] + rank // P
    p_idx = rank % P

    in_maps = []
    for c in range(N_CORES):
        m = core_s == c
        # oh[p, t, 0:128] = vcoh (value one-hot over c128, pre-scaled)
        # oh[p, t, 128:256] = roh (one-hot over r128)
        oh = np.zeros((P, t_total, 2 * P), dtype=ml_dtypes.float8_e3m4)
        oh[p_idx[m], t_idx[m], c128[order][m]] = vals[order][m]
        oh[p_idx[m], t_idx[m], P + r128[order][m]] = 1.0
        wt = np.ascontiguousarray(
            w[c * O_SHARD : (c + 1) * O_SHARD, :].T.reshape(NK, P, O_SHARD)
            .transpose(1, 0, 2)
        ).astype(ml_dtypes.bfloat16)
        bias_rep = np.broadcast_to(
            bias_np[c * O_SHARD : (c + 1) * O_SHARD], (P, O_SHARD)
        ).copy()
        in_maps.append({"xt": xt, "wt": wt, "bias": bias_rep, "oh": oh})
    return bucket_tiles, in_maps


def kernel(x, weight, bias, shira_weight, shira_indices, _trace=False):
    bucket_tiles, in_maps = _prep_inputs(
        x, weight, bias, shira_weight, shira_indices
    )
    nc = _build_bass(bucket_tiles)
    _split_multi_waits(nc)
    res = run_bass_kernel_spmd(
        nc, in_maps, core_ids=list(range(N_CORES)), trace=_trace
    )
    out = np.concatenate([r["out"] for r in res.results], axis=1)
    out = out.reshape(4, 2048, OUT_F)
    if _trace:
        kernel.last_results = res
    return out
